# revision 33
# baseline (speedup 1.0000x reference)
"""Distributed Trainium2 Bass kernel for the AGCN (gnn_message_passing) problem.

Strategy (8 NeuronCores, SPMD):
  - Nodes partitioned by graph id: core c owns graphs [8c, 8c+8) and their
    nodes in natural (batch-sorted) order, so graphs are contiguous.
  - Edges (no self loops) are owned by the dst node's core, grouped by dst
    128-block; per-block tile counts are the max over cores (SPMD-uniform).
  - Per layer: each core computes hW = feat @ W per block (PE), writes its
    [R,1024] bf16 shard, and the shard is AllGathered in NCH row-chunks so
    the collective overlaps the aggregation phase of the previous blocks.
    Aggregation per dst block: dma_gather of src rows (edge-major [128e,1024])
    from the gathered table + TensorE scatter-matmul psum += S^T @ msg with a
    host-built one-hot*enorm S tile. Self-loop term is applied on DVE
    (hW[b] * self_norm) from a sequential shard read-back; bias via a K=1
    matmul. Epilogue: relu (+ residual), then feature-major transposes feed
    next layer's hW immediately (per-block pipelining).
  - Pooling: final feat written to DRAM per block; one transposed dma_gather
    per graph yields feature-major columns; DVE max-reduce. Padding gathers
    an all-zero row, which never beats the max of post-relu (>=0) features.
  - Readout MLP per-core on its own 8 graphs; host concatenates.
"""
import os
import sys

for _p in ("/opt/trn_rl_repo",):
    if os.path.isdir(_p) and _p not in sys.path:
        sys.path.insert(0, _p)

import numpy as np
import ml_dtypes

from concourse import bass, bacc, tile
import concourse.mybir as mybir
from concourse.bass_utils import run_bass_kernel_spmd

BF = ml_dtypes.bfloat16
F8 = ml_dtypes.float8_e4m3
NCORES = 8
D = 512
DE = 1280
NEMB = 21
NCH = 2          # AllGather chunks per layer
GPC = 8          # graphs per core


# ---------------------------------------------------------------------------
# host-side preprocessing
# ---------------------------------------------------------------------------

def _wrap_idx(a):
    """[n] int16 -> [128, n//16] wrapped (idx i at partition i%16, col i//16),
    replicated across the 8 Q7 core groups."""
    a = np.ascontiguousarray(np.asarray(a, np.int16)).reshape(-1, 16).T
    return np.ascontiguousarray(np.tile(a, (8, 1)))


def _prep(inputs):
    native_x = np.asarray(inputs["native_x"], np.int64)
    x = np.asarray(inputs["x"], np.float32)
    edge_index = np.asarray(inputs["edge_index"], np.int64)
    batch = np.asarray(inputs["batch"], np.int64)
    N = native_x.shape[0]
    G = 64

    src, dst = edge_index[0], edge_index[1]
    deg = np.bincount(dst, minlength=N).astype(np.float32) + 1.0
    dis = deg ** -0.5
    enorm = (dis[src] * dis[dst]).astype(np.float32)
    self_norm = (dis * dis).astype(np.float32)

    core_of_node = batch // GPC
    counts = np.bincount(core_of_node, minlength=NCORES)
    cstart = np.concatenate([[0], np.cumsum(counts)])
    R = int(np.ceil(counts.max() / 128) * 128)
    B = R // 128
    loc = np.arange(N) - cstart[core_of_node]          # natural within-core rank

    # AllGather chunk schedule (in blocks)
    chunks = [(int(a[0]), int(a[-1]) + 1) for a in np.array_split(np.arange(B), NCH)]
    # table row index (chunk-major: [chunk][core][rows-within-chunk])
    blk = loc // 128
    ch_of_blk = np.zeros(B, np.int64)
    for j, (b0, b1) in enumerate(chunks):
        ch_of_blk[b0:b1] = j
    ch = ch_of_blk[blk]
    cb0 = np.array([chunks[j][0] for j in range(NCH)])
    cb1 = np.array([chunks[j][1] for j in range(NCH)])
    rc = (cb1 - cb0) * 128                              # rows per core per chunk
    pg = 8 * 128 * cb0[ch] + core_of_node * rc[ch] + (loc - 128 * cb0[ch])
    assert NCORES * R < 32768

    # edges grouped by (dst core, dst block)
    key = core_of_node[dst] * B + blk[dst]
    order = np.argsort(key, kind="stable")
    skey = key[order]
    starts = np.searchsorted(skey, np.arange(NCORES * B))
    ends = np.searchsorted(skey, np.arange(NCORES * B) + 1)
    cnt = (ends - starts).reshape(NCORES, B)
    tb = np.maximum(np.ceil(cnt.max(axis=0) / 128).astype(np.int64), 1)  # [B]
    ot = np.concatenate([[0], np.cumsum(tb)])           # tile offsets
    TT = int(ot[-1])

    gsz = np.bincount(batch, minlength=G)
    assert gsz.min() > 0, "empty graph unsupported"
    SLOT = int(np.ceil(gsz.max() / 128) * 128)
    gstart = np.concatenate([[0], np.cumsum(gsz)])
    # per local-graph-slot conservative block windows (uniform across cores)
    w0 = [min((gstart[c * GPC + j] - cstart[c]) // 128
              for c in range(NCORES)) for j in range(GPC)]
    w1 = [max((gstart[c * GPC + j + 1] - 1 - cstart[c]) // 128 + 1
              for c in range(NCORES)) for j in range(GPC)]

    cores = []
    for c in range(NCORES):
        gidx = np.zeros(TT * 128, np.int16)
        S = np.zeros((TT * 128, 128), F8)
        for b in range(B):
            sl = order[starts[c * B + b]:ends[c * B + b]]
            k = len(sl)
            if k:
                gidx[ot[b] * 128:ot[b] * 128 + k] = pg[src[sl]].astype(np.int16)
                S[ot[b] * 128 + np.arange(k), loc[dst[sl]] % 128] = enorm[sl].astype(F8)
        n0, n1 = cstart[c], cstart[c + 1]
        nn = n1 - n0
        xT = np.zeros((DE, R), np.float32)
        xT[:, :nn] = x[n0:n1].T
        oh = np.zeros((NEMB, R), BF)
        oh[native_x[n0:n1], np.arange(nn)] = 1.0
        # self-loop diag tiles: sd[i, b*128+i] = self_norm of node b*128+i
        sd = np.zeros((128, B * 128), BF)
        sd[np.arange(nn) % 128, np.arange(nn)] = self_norm[n0:n1].astype(BF)
        # pooling slot indices per graph, relative to its block window;
        # padding repeats the graph's first node (duplicates don't move a max)
        sidx = np.zeros(GPC * SLOT, np.int64)
        for j in range(GPC):
            g = c * GPC + j
            base = w0[j] * 128
            sidx[j * SLOT:(j + 1) * SLOT] = gstart[g] - n0 - base
            sidx[j * SLOT:j * SLOT + gsz[g]] = (
                np.arange(gstart[g], gstart[g + 1]) - n0 - base)

        st_dev = np.ascontiguousarray(
            S.reshape(TT, 128, 128).transpose(1, 0, 2).reshape(128, TT * 128))
        xT_dev = np.ascontiguousarray(
            xT.reshape(10, 128, R).transpose(1, 0, 2).reshape(128, 10 * R)).astype(BF)
        cores.append(dict(stiles=st_dev, gidx=_wrap_idx(gidx), sidx=_wrap_idx(sidx),
                          xT=xT_dev, oh=np.ascontiguousarray(oh),
                          selfd=np.ascontiguousarray(sd)))
    return dict(cores=cores, R=R, B=B, TT=TT, tb=tuple(int(t) for t in tb),
                chunks=tuple(chunks), SLOT=SLOT,
                wins=tuple(zip((int(v) for v in w0), (int(v) for v in w1))))


def _params(inputs, dims):
    """Parameter tensors (identical on every core)."""
    emb = np.asarray(inputs["embed_table"], np.float32)
    aaw = np.asarray(inputs["proj_aa_w"], np.float32)
    p = {}
    p["aa_tab"] = np.ascontiguousarray((emb @ aaw).astype(BF))            # [21, 512]
    wesm = np.asarray(inputs["proj_esm_w"], np.float32).astype(BF)        # [1280,512]
    p["wesm"] = np.ascontiguousarray(
        wesm.reshape(10, 128, D).transpose(1, 0, 2).reshape(128, 10 * D))
    gw = np.asarray(inputs["gcn_w"], np.float32).astype(BF)               # [3,512,512]
    p["gcnw"] = np.ascontiguousarray(
        gw.reshape(3, 4, 128, D).transpose(2, 0, 1, 3).reshape(128, 12 * D))
    p["b_esm"] = np.asarray(inputs["proj_esm_b"], np.float32).astype(BF).reshape(1, D)
    p["b_aa"] = np.asarray(inputs["proj_aa_b"], np.float32).astype(BF).reshape(1, D)
    gb = np.asarray(inputs["gcn_b"], np.float32).astype(BF)
    p["gcnb"] = np.ascontiguousarray(np.tile(gb, (1, 2)).reshape(1, -1)) # [1, 3*1024]
    r1 = np.asarray(inputs["ro1_w"], np.float32).astype(BF)               # [512,1024]
    p["ro1w"] = np.ascontiguousarray(
        r1.reshape(4, 128, 1024).transpose(1, 0, 2).reshape(128, 4 * 1024))
    p["ro1b"] = np.asarray(inputs["ro1_b"], np.float32).astype(BF).reshape(1, 1024)
    r2 = np.asarray(inputs["ro2_w"], np.float32).astype(BF)               # [1024,500]
    p["ro2w"] = np.ascontiguousarray(
        r2.reshape(8, 128, 500).transpose(1, 0, 2).reshape(128, 8 * 500))
    p["ro2b"] = np.asarray(inputs["ro2_b"], np.float32).astype(BF).reshape(1, 500)
    w1 = np.asarray(inputs["weight1"], np.float32)
    p["w1a"] = np.full((128, 1), w1[0], np.float32)
    p["w1b"] = np.full((128, 1), w1[1], np.float32)
    p["ident"] = np.eye(128, dtype=BF)
    p["ident8"] = np.eye(8, dtype=BF)
    p["ones"] = np.ones((1, 128), BF)
    return p


# ---------------------------------------------------------------------------
# device kernel builder
# ---------------------------------------------------------------------------

def _build(dims):
    R, B, TT, tb, chunks, SLOT = (dims["R"], dims["B"], dims["TT"],
                                  dims["tb"], dims["chunks"], dims["SLOT"])
    wins = dims["wins"]
    ot = [0]
    for t in tb:
        ot.append(ot[-1] + t)
    TBMAX = max(tb)
    f32, bf16, i16 = mybir.dt.float32, mybir.dt.bfloat16, mybir.dt.int16
    f8 = mybir.dt.float8e4
    RELU = mybir.ActivationFunctionType.Relu
    SIGM = mybir.ActivationFunctionType.Sigmoid
    MULT = mybir.AluOpType.mult
    chunk_end = {b1 - 1: j for j, (b0, b1) in enumerate(chunks)}

    nc = bacc.Bacc(None, target_bir_lowering=False, debug=False)

    # I/O
    d_xT = nc.declare_dram_parameter("xT", [128, 10 * R], bf16, isOutput=False)
    d_oh = nc.declare_dram_parameter("oh", [NEMB, R], bf16, isOutput=False)
    d_st = nc.declare_dram_parameter("stiles", [128, TT * 128], f8, isOutput=False)
    d_gidx = nc.declare_dram_parameter("gidx", [128, TT * 8], i16, isOutput=False)
    d_sidx = nc.declare_dram_parameter("sidx", [128, GPC * SLOT // 16], i16, isOutput=False)
    d_sd = nc.declare_dram_parameter("selfd", [128, B * 128], bf16, isOutput=False)
    d_aatab = nc.declare_dram_parameter("aa_tab", [NEMB, D], bf16, isOutput=False)
    d_wesm = nc.declare_dram_parameter("wesm", [128, 10 * D], bf16, isOutput=False)
    d_gcnw = nc.declare_dram_parameter("gcnw", [128, 12 * D], bf16, isOutput=False)
    d_besm = nc.declare_dram_parameter("b_esm", [1, D], bf16, isOutput=False)
    d_baa = nc.declare_dram_parameter("b_aa", [1, D], bf16, isOutput=False)
    d_gcnb = nc.declare_dram_parameter("gcnb", [1, 3 * 1024], bf16, isOutput=False)
    d_ro1w = nc.declare_dram_parameter("ro1w", [128, 4 * 1024], bf16, isOutput=False)
    d_ro1b = nc.declare_dram_parameter("ro1b", [1, 1024], bf16, isOutput=False)
    d_ro2w = nc.declare_dram_parameter("ro2w", [128, 8 * 500], bf16, isOutput=False)
    d_ro2b = nc.declare_dram_parameter("ro2b", [1, 500], bf16, isOutput=False)
    d_w1a = nc.declare_dram_parameter("w1a", [128, 1], f32, isOutput=False)
    d_w1b = nc.declare_dram_parameter("w1b", [128, 1], f32, isOutput=False)
    d_ident = nc.declare_dram_parameter("ident", [128, 128], bf16, isOutput=False)
    d_ident8 = nc.declare_dram_parameter("ident8", [8, 8], bf16, isOutput=False)
    d_ones = nc.declare_dram_parameter("ones", [1, 128], bf16, isOutput=False)
    d_out = nc.declare_dram_parameter("out", [GPC, 500], f32, isOutput=True)

    # internal DRAM
    shards = [nc.dram_tensor(f"hw_shard{i}", [R, 1024], f8) for i in range(2)]
    tables = [nc.dram_tensor(f"table{l}", [NCORES * R, 1024], f8,
                             addr_space="Shared") for l in range(3)]
    ffin = nc.dram_tensor("feat_final", [R, 1024], bf16)
    d_wi = nc.dram_tensor("warm_in", [128, 64], f8)
    d_wo = nc.dram_tensor("warm_out", [NCORES * 128, 64], f8, addr_space="Shared")

    with tile.TileContext(nc) as tc:
        with (
            tc.tile_pool(name="persist", bufs=1) as pers,
            tc.tile_pool(name="feat", bufs=1) as featp,
        ):
            # warmup collective: absorbs the replica barrier + ncfw cold start
            # under the input-projection phase (no data deps)
            nc.gpsimd.collective_compute(
                "AllGather", mybir.AluOpType.bypass,
                replica_groups=[list(range(NCORES))],
                ins=[d_wi.ap().opt()], outs=[d_wo.ap().opt()],
            )
            # persistent params in SBUF
            s_st = pers.tile([128, TT * 128], f8)
            nc.sync.dma_start(s_st[:], d_st.ap())
            s_sd = pers.tile([128, B * 128], bf16)
            nc.sync.dma_start(s_sd[:], d_sd.ap())
            s_gidx = pers.tile([128, TT * 8], i16)
            nc.sync.dma_start(s_gidx[:], d_gidx.ap())
            s_gcnw = pers.tile([128, 12, D], bf16)
            nc.sync.dma_start(s_gcnw[:], d_gcnw.ap().rearrange("p (a d) -> p a d", d=D))
            s_gcnb = pers.tile([1, 3, 1024], bf16)
            nc.sync.dma_start(s_gcnb[:], d_gcnb.ap().rearrange("p (a d) -> p a d", d=1024))
            s_ident = pers.tile([128, 128], bf16)
            nc.sync.dma_start(s_ident[:], d_ident.ap())
            s_ones = pers.tile([1, 128], bf16)
            nc.sync.dma_start(s_ones[:], d_ones.ap())

            # pool/readout params, preloaded so the tail never waits on DMA
            s_sidx = pers.tile([128, GPC * SLOT // 16], i16)
            nc.sync.dma_start(s_sidx[:], d_sidx.ap())
            s_w1a = pers.tile([128, 1], f32)
            nc.sync.dma_start(s_w1a[:], d_w1a.ap())
            s_w1b = pers.tile([128, 1], f32)
            nc.sync.dma_start(s_w1b[:], d_w1b.ap())
            s_ro1w = pers.tile([128, 4, 1024], bf16)
            nc.sync.dma_start(s_ro1w[:], d_ro1w.ap().rearrange("p (a d) -> p a d", d=1024))
            s_ro1b = pers.tile([1, 1024], bf16)
            nc.sync.dma_start(s_ro1b[:], d_ro1b.ap())
            s_ro2w = pers.tile([128, 8, 500], bf16)
            nc.sync.dma_start(s_ro2w[:], d_ro2w.ap().rearrange("p (a d) -> p a d", d=500))
            s_ro2b = pers.tile([1, 500], bf16)
            nc.sync.dma_start(s_ro2b[:], d_ro2b.ap())
            s_id8 = pers.tile([8, 8], bf16)
            nc.sync.dma_start(s_id8[:], d_ident8.ap())

            featA = featp.tile([128, B, D], bf16, tag="featA")
            featB = featp.tile([128, B, D], bf16, tag="featB")
            hwbuf = featp.tile([128, B, 1024], bf16, tag="hwbuf")
            st3 = s_st[:].rearrange("p (bt n) -> p bt n", n=128)

            def transposes_and_hw(l, b, ftp, pst, psh, hwp):
                """feature-major transposes of feat[b] + hW_{l} for block b,
                written to shards[l%2]. Returns the ft tiles."""
                fts = []
                for feat in (featA, featB):
                    ftps = pst.tile([128, D], f32, tag="ftps")
                    for k in range(4):
                        nc.tensor.matmul(ftps[:, k * 128:(k + 1) * 128],
                                         feat[:, b, k * 128:(k + 1) * 128],
                                         s_ident[:], start=True, stop=True)
                    ft = ftp.tile([128, D], bf16, tag="ft")
                    nc.vector.tensor_copy(ft[:], ftps[:])
                    fts.append(ft)
                if l > 2:
                    return fts
                hwps = psh.tile([128, 1024], f32, tag="hwps")
                for s, ft in enumerate(fts):
                    for k in range(4):
                        nc.tensor.matmul(hwps[:, s * D:(s + 1) * D],
                                         ft[:, k * 128:(k + 1) * 128],
                                         s_gcnw[:, l * 4 + k, :],
                                         start=(k == 0), stop=(k == 3))
                nc.vector.tensor_copy(hwbuf[:, b, :], hwps[:])
                hw8 = hwp.tile([128, 1024], f8, tag="hw8")
                nc.scalar.activation(hw8[:], hwps[:],
                                     mybir.ActivationFunctionType.Copy)
                nc.sync.dma_start(shards[l % 2].ap()[b * 128:(b + 1) * 128, :], hw8[:])
                return fts

            def ag_chunk(l, j):
                b0, b1 = chunks[j]
                nc.gpsimd.collective_compute(
                    "AllGather", mybir.AluOpType.bypass,
                    replica_groups=[list(range(NCORES))],
                    ins=[shards[l % 2].ap()[b0 * 128:b1 * 128, :].opt()],
                    outs=[tables[l].ap()[8 * b0 * 128:8 * b1 * 128, :].opt()],
                )

            # ---------------- input projection + hW_0 ----------------
            with (
                tc.tile_pool(name="xin", bufs=3) as xin,
                tc.tile_pool(name="prm1", bufs=1) as prm1,
                tc.tile_pool(name="pft0", bufs=2) as ftp0,
                tc.tile_pool(name="phw0", bufs=2) as hwp0,
                tc.tile_pool(name="ps1", bufs=2, space="PSUM") as pps1,
                tc.tile_pool(name="ps2", bufs=2, space="PSUM") as pps2,
                tc.tile_pool(name="pst0", bufs=2, space="PSUM") as pst0,
                tc.tile_pool(name="psh0", bufs=1, space="PSUM") as psh0,
            ):
                s_wesm = prm1.tile([128, 10, D], bf16)
                nc.sync.dma_start(s_wesm[:], d_wesm.ap().rearrange("p (a d) -> p a d", d=D))
                s_oh = prm1.tile([NEMB, R], bf16)
                nc.sync.dma_start(s_oh[:], d_oh.ap())
                s_aatab = prm1.tile([NEMB, D], bf16)
                nc.sync.dma_start(s_aatab[:], d_aatab.ap())
                s_besm = prm1.tile([1, D], bf16)
                nc.sync.dma_start(s_besm[:], d_besm.ap())
                s_baa = prm1.tile([1, D], bf16)
                nc.sync.dma_start(s_baa[:], d_baa.ap())

                xT3 = d_xT.ap().rearrange("p (a r) -> p a r", r=R)
                for b in range(B):
                    xt = xin.tile([128, 10, 128], bf16, tag="xt")
                    nc.sync.dma_start(xt[:], xT3[:, :, b * 128:(b + 1) * 128])
                    ps1 = pps1.tile([128, D], f32, tag="ps1")
                    for k in range(10):
                        nc.tensor.matmul(ps1[:], xt[:, k, :], s_wesm[:, k, :],
                                         start=(k == 0), stop=False)
                    nc.tensor.matmul(ps1[:], s_ones[:], s_besm[:],
                                     start=False, stop=True)
                    ps2 = pps2.tile([128, D], f32, tag="ps2")
                    nc.tensor.matmul(ps2[:], s_oh[:, b * 128:(b + 1) * 128],
                                     s_aatab[:], start=True, stop=False)
                    nc.tensor.matmul(ps2[:], s_ones[:], s_baa[:],
                                     start=False, stop=True)
                    lin = xin.tile([128, D], f32, tag="lin")
                    nc.vector.tensor_copy(lin[:], ps1[:])
                    nc.scalar.activation(featB[:, b, :], ps1[:], RELU)
                    nc.vector.tensor_add(ps2[:], ps2[:], lin[:])
                    nc.scalar.activation(featA[:, b, :], ps2[:], RELU)
                    transposes_and_hw(0, b, ftp0, pst0, psh0, hwp0)
                    if b in chunk_end:
                        ag_chunk(0, chunk_end[b])

            # ---------------- GCN layers ----------------
            for l in range(3):
                with (
                    tc.tile_pool(name=f"ft{l}", bufs=2) as ftp,
                    tc.tile_pool(name=f"hw{l}", bufs=2) as hwp,
                    tc.tile_pool(name=f"gb{l}", bufs=3) as gbp,
                    tc.tile_pool(name=f"ep{l}", bufs=2) as epp,
                    tc.tile_pool(name=f"pst{l}", bufs=2, space="PSUM") as pst,
                    tc.tile_pool(name=f"psh{l}", bufs=1, space="PSUM") as psh,
                    tc.tile_pool(name=f"psa{l}", bufs=2, space="PSUM") as psa,
                ):
                    for b in range(B):
                        t0b, t1b = ot[b], ot[b + 1]
                        nt_all = t1b - t0b
                        gbuf = gbp.tile([128, TBMAX, 1024], f8, tag="gbuf")
                        for t0 in range(0, nt_all, 8):
                            nt = min(8, nt_all - t0)
                            nc.gpsimd.dma_gather(
                                gbuf[:, t0:t0 + nt, :], tables[l].ap(),
                                s_gidx[:, (t0b + t0) * 8:(t0b + t0 + nt) * 8],
                                nt * 128, nt * 128, 1024)
                        aps = psa.tile([128, 1024], f32, tag="aps")
                        DR = mybir.MatmulPerfMode.DoubleRow
                        for t in range(0, nt_all - 1, 2):
                            for h in range(2):
                                nc.tensor.matmul(
                                    aps[:, h * D:(h + 1) * D],
                                    st3[:, t0b + t:t0b + t + 2, :],
                                    gbuf[:, t:t + 2, h * D:(h + 1) * D],
                                    start=(t == 0), stop=False, perf_mode=DR)
                        if nt_all % 2:
                            t = nt_all - 1
                            for h in range(2):
                                nc.tensor.matmul(
                                    aps[:, h * D:(h + 1) * D],
                                    st3[:, t0b + t, :],
                                    gbuf[:, t, h * D:(h + 1) * D],
                                    start=(t == 0), stop=False)
                        for h in range(2):
                            nc.tensor.matmul(
                                aps[:, h * D:(h + 1) * D],
                                s_sd[:, b * 128:(b + 1) * 128],
                                hwbuf[:, b, h * D:(h + 1) * D],
                                start=False, stop=False)
                            nc.tensor.matmul(aps[:, h * D:(h + 1) * D],
                                             s_ones[:],
                                             s_gcnb[:, l, h * D:(h + 1) * D],
                                             start=False, stop=True)
                        if l == 0:
                            nc.scalar.activation(featA[:, b, :], aps[:, :D], RELU)
                            nc.scalar.activation(featB[:, b, :], aps[:, D:], RELU)
                        else:
                            ra = epp.tile([128, D], bf16, tag="ra")
                            rb = epp.tile([128, D], bf16, tag="rb")
                            nc.scalar.activation(ra[:], aps[:, :D], RELU)
                            nc.scalar.activation(rb[:], aps[:, D:], RELU)
                            nc.vector.tensor_add(featA[:, b, :], featA[:, b, :], ra[:])
                            nc.vector.tensor_add(featB[:, b, :], featB[:, b, :], rb[:])
                        if l == 2:
                            nc.sync.dma_start(
                                ffin.ap()[b * 128:(b + 1) * 128, :D], featA[:, b, :])
                            nc.sync.dma_start(
                                ffin.ap()[b * 128:(b + 1) * 128, D:], featB[:, b, :])
                        else:
                            transposes_and_hw(l + 1, b, ftp, pst, psh, hwp)
                            if b in chunk_end:
                                ag_chunk(l + 1, chunk_end[b])

            # ---------------- pooling + readout ----------------
            with (
                tc.tile_pool(name="slot", bufs=2) as slotp,
                tc.tile_pool(name="prm2", bufs=1) as prm2,
                tc.tile_pool(name="pps", bufs=2, space="PSUM") as pps,
                tc.tile_pool(name="rps", bufs=1, space="PSUM") as rps,
            ):
                pooled = prm2.tile([128, 8, GPC], f32)
                for j in range(GPC):
                    b0, b1 = wins[j]
                    sbuf = slotp.tile([128, 8, SLOT], bf16, tag="sbuf")
                    nc.gpsimd.dma_gather(
                        sbuf[:], ffin.ap()[b0 * 128:b1 * 128, :],
                        s_sidx[:, j * (SLOT // 16):(j + 1) * (SLOT // 16)],
                        SLOT, SLOT, 1024, transpose=True)
                    for kb in range(8):
                        nc.vector.tensor_reduce(
                            pooled[:, kb, j:j + 1], sbuf[:, kb, :],
                            axis=mybir.AxisListType.X, op=mybir.AluOpType.max)
                gT = prm2.tile([128, 4, GPC], bf16)
                gtmp = prm2.tile([128, 4, GPC], f32)
                gtmp2 = prm2.tile([128, 4, GPC], f32)
                nc.vector.tensor_scalar(gtmp[:], pooled[:, 0:4, :], s_w1a[:], None, MULT)
                nc.vector.tensor_scalar(gtmp2[:], pooled[:, 4:8, :], s_w1b[:], None, MULT)
                nc.vector.tensor_add(gT[:], gtmp[:], gtmp2[:])

                r1ps = rps.tile([GPC, 1024], f32)
                for h in range(2):
                    for k in range(4):
                        nc.tensor.matmul(r1ps[:, h * D:(h + 1) * D],
                                         gT[:, k, :],
                                         s_ro1w[:, k, h * D:(h + 1) * D],
                                         start=(k == 0), stop=False)
                    nc.tensor.matmul(r1ps[:, h * D:(h + 1) * D],
                                     s_ones[:, :GPC],
                                     s_ro1b[:, h * D:(h + 1) * D],
                                     start=False, stop=True)
                h1 = prm2.tile([GPC, 1024], bf16)
                nc.scalar.activation(h1[:], r1ps[:], RELU)
                h1T = prm2.tile([128, 8, GPC], bf16)
                for k in range(8):
                    tps = pps.tile([128, GPC], f32, tag="tps8")
                    nc.tensor.matmul(tps[:], h1[:, k * 128:(k + 1) * 128],
                                     s_id8[:], start=True, stop=True)
                    nc.vector.tensor_copy(h1T[:, k, :], tps[:])
                yps = rps.tile([GPC, 500], f32)
                for k in range(8):
                    nc.tensor.matmul(yps[:], h1T[:, k, :], s_ro2w[:, k, :],
                                     start=(k == 0), stop=False)
                nc.tensor.matmul(yps[:], s_ones[:, :GPC], s_ro2b[:],
                                 start=False, stop=True)
                y = prm2.tile([GPC, 500], f32)
                nc.scalar.activation(y[:], yps[:], SIGM)
                nc.sync.dma_start(d_out.ap(), y[:])

    nc.compile()
    return nc


# ---------------------------------------------------------------------------
# entry point
# ---------------------------------------------------------------------------

_CACHE = {}


def kernel(**inputs):
    pp = _prep(inputs)
    dims = {k: pp[k] for k in ("R", "B", "TT", "tb", "chunks", "SLOT", "wins")}
    key = (dims["R"], dims["TT"], dims["tb"], dims["chunks"], dims["SLOT"],
           dims["wins"])
    if key not in _CACHE:
        _CACHE[key] = _build(dims)
    nc = _CACHE[key]
    par = _params(inputs, dims)
    in_maps = []
    for c in range(NCORES):
        m = dict(par)
        m.update(pp["cores"][c])
        in_maps.append(m)
    res = run_bass_kernel_spmd(nc, in_maps, core_ids=list(range(NCORES)))
    out = np.concatenate([res.results[c]["out"] for c in range(NCORES)], 0)
    return out.astype(np.float32)
